# revision 33
# baseline (speedup 1.0000x reference)
"""Trainium2 Bass kernel for MLA self-attention (nn_MLASelfAttentionWithMoBA).

Sharding: 8 cores = 4 batches x 2 head-halves (tensor parallel over heads).
Each core handles one batch element and 8 of the 16 heads:
  - down-projections x @ W*_down restricted to its heads' latent columns
  - per-head up-projections + partial RoPE (pair-mixing folded into a
    host-precomputed "rotated" copy of the up-projection weights)
  - full causal attention for its 8 heads, scores kept transposed [k, q]
    so A@V needs no P transposes; softmax denominators come from an extra
    ones-column appended to V; normalization is applied to the attention
    output before the c_proj
  - row-parallel c_proj with its 512 rows of Wc -> partial y
Host combines: y = y_half0 + y_half1 per batch; k_lat/v_lat are written
head-major-transposed [512, T] per core and transposed on the host.

Down-projection runs in f32r (tf32-like, full PE rate) so the k_lat/v_lat
outputs keep ~1e-4 precision; attention/up/c_proj operands are bf16/f32r.
"""

import functools

import numpy as np

import concourse.bass as bass
import concourse.tile as tile
from concourse import bacc, mybir
from concourse.bass_utils import run_bass_kernel_spmd

F32 = mybir.dt.float32
F32R = mybir.dt.float32r
BF16 = mybir.dt.bfloat16

B, T, C, H, L = 4, 1024, 1024, 16, 64
DH, DE = 64, 32
NCORES = 8
HPC = H // 2  # heads per core (8)
PAIRS = HPC // 2  # head pairs per core (4)
KT = T // 128  # key tiles (8)
QC = T // 512  # q chunks (2)

TRACE = False  # set by test.py to capture an NTFF profile


def _rope_tables():
    inv = 1.0 / (10000.0 ** (np.arange(0, DE, 2, dtype=np.float64) / DE))  # [16]
    t = np.arange(T, dtype=np.float64)
    freqs = np.outer(t, inv)  # [T, 16]
    emb = np.concatenate([freqs, freqs], axis=-1)  # [T, 32]
    cos = np.cos(emb).T.astype(np.float32)  # [32, T]
    sin = np.sin(emb).T.astype(np.float32)
    return cos, sin


def _rot_cols(w):
    """w: [L, DE]. Columns permuted/negated so lat@w_rot == rotate_half(lat@w)."""
    r = np.empty_like(w)
    r[:, 0::2] = -w[:, 1::2]
    r[:, 1::2] = w[:, 0::2]
    return r


def _blockdiag2(a):
    z = np.zeros_like(a)
    return np.block([[a, z], [z, a]])  # [2m, 2n]


def _rot_ext(w):
    """[128, 128] table: out rows 32-63 <- rot-proj(h0), rows 96-127 <- rot-proj(h1),
    other out rows 0 (so the rope add is one full-height op)."""
    r = _rot_cols(w)  # [64, 32]
    ext = np.zeros((128, 128), dtype=np.float32)
    ext[0:64, 32:64] = r
    ext[64:128, 96:128] = r
    return ext


def _build_nc():
    nc = bacc.Bacc("TRN2", target_bir_lowering=False, debug=False, num_devices=NCORES)

    x_d = nc.dram_tensor("x", [T, C], F32, kind="ExternalInput")
    wq_d = nc.dram_tensor("wq", [C, 512], BF16, kind="ExternalInput")
    wk_d = nc.dram_tensor("wk", [C, 512], BF16, kind="ExternalInput")
    wv_d = nc.dram_tensor("wv", [C, 512], BF16, kind="ExternalInput")
    wc_d = nc.dram_tensor("wc", [512, C], BF16, kind="ExternalInput")
    bd_ce_q_d = nc.dram_tensor("bd_ce_q", [128, 128], BF16, kind="ExternalInput")
    bd_ce_k_d = nc.dram_tensor("bd_ce_k", [128, 128], BF16, kind="ExternalInput")
    bd_rot_q_d = nc.dram_tensor("bd_rot_q", [128, 128], BF16, kind="ExternalInput")
    bd_rot_k_d = nc.dram_tensor("bd_rot_k", [128, 128], BF16, kind="ExternalInput")
    bd_v_d = nc.dram_tensor("bd_v", [128, 128], BF16, kind="ExternalInput")
    cc_d = nc.dram_tensor("cc", [128, T], BF16, kind="ExternalInput")
    ss_d = nc.dram_tensor("ss", [128, T], BF16, kind="ExternalInput")
    tri2_d = nc.dram_tensor("tri2", [128, 2, 128], BF16, kind="ExternalInput")
    tmask_d = nc.dram_tensor("tmask", [128, 128], BF16, kind="ExternalInput")
    idbf_d = nc.dram_tensor("idbf", [128, 128], BF16, kind="ExternalInput")
    ident_d = nc.dram_tensor("ident", [128, 128], F32, kind="ExternalInput")

    y_d = nc.dram_tensor("y_part", [T, C], F32, kind="ExternalOutput")
    klat_d = nc.dram_tensor("klatT", [512, T], F32, kind="ExternalOutput")
    vlat_d = nc.dram_tensor("vlatT", [512, T], F32, kind="ExternalOutput")

    with tile.TileContext(nc) as tc:
        with tc.tile_pool(name="const", bufs=1) as const, tc.tile_pool(
            name="persist", bufs=1
        ) as persist, tc.tile_pool(
            name="psA", bufs=2, space="PSUM"
        ) as psA, tc.tile_pool(name="psO", bufs=1, space="PSUM") as psO:

            def pa(shape, name):
                padded = [128] + list(shape[1:-1]) + [T // (int(np.prod(shape[1:-1])) or 1)]
                return psA.tile(shape, F32, tag="A", name=name, padded_shape=padded)

            # ---- constants ----
            cc_t = const.tile([128, T], BF16, tag="cc")
            nc.sync.dma_start(out=cc_t[:], in_=cc_d[:])
            ss_t = const.tile([128, T], BF16, tag="ss")
            nc.sync.dma_start(out=ss_t[:], in_=ss_d[:])
            tri2_t = const.tile([128, 2, 128], BF16, tag="tri2")
            nc.sync.dma_start(out=tri2_t[:], in_=tri2_d[:])
            id_t = const.tile([128, 128], F32, tag="ident")
            nc.sync.dma_start(out=id_t[:], in_=ident_d[:])
            bd_ce_q = const.tile([128, 128], BF16, tag="bd_ce_q")
            nc.sync.dma_start(out=bd_ce_q[:], in_=bd_ce_q_d[:])
            bd_ce_k = const.tile([128, 128], BF16, tag="bd_ce_k")
            nc.sync.dma_start(out=bd_ce_k[:], in_=bd_ce_k_d[:])
            bd_rot_q = const.tile([128, 128], BF16, tag="bd_rot_q")
            nc.sync.dma_start(out=bd_rot_q[:], in_=bd_rot_q_d[:])
            bd_rot_k = const.tile([128, 128], BF16, tag="bd_rot_k")
            nc.sync.dma_start(out=bd_rot_k[:], in_=bd_rot_k_d[:])
            bd_v_t = const.tile([128, 128], BF16, tag="bd_v")
            nc.sync.dma_start(out=bd_v_t[:], in_=bd_v_d[:])
            tmask_t = const.tile([128, 128], BF16, tag="tmask")
            nc.sync.dma_start(out=tmask_t[:], in_=tmask_d[:])
            idbf_t = const.tile([128, 128], BF16, tag="idbf")
            nc.sync.dma_start(out=idbf_t[:], in_=idbf_d[:])

            # PE warmup burst so the HAM clock-gate reaches K=8 early
            wsrc = const.tile([128, 512], BF16, tag="wsrc")
            nc.vector.memset(wsrc[:], 1.0)
            for wi in range(8):
                pw = pa([128, 512], "pw")
                nc.tensor.matmul(pw[:], wsrc[:, 0:128], wsrc[:], start=True, stop=True)

            # Wc: load early so c_proj never waits on it
            wcr = []
            for fp in range(PAIRS):
                wt = persist.tile([128, C], BF16, tag=f"wcr{fp}", name=f"wcr{fp}")
                nc.scalar.dma_start(out=wt[:], in_=wc_d[fp * 128 : (fp + 1) * 128, :])
                wcr.append(wt)

            # persistent intermediates (bf16 to fit SBUF)
            qlat_t = persist.tile([128, PAIRS, T], BF16, tag="qlat")
            klat_t = persist.tile([128, PAIRS, T], BF16, tag="klat")
            vlat_t = persist.tile([128, PAIRS, T], BF16, tag="vlat")
            v_g = persist.tile([128, KT, HPC, 65], BF16, tag="v_g")
            kT = [persist.tile([128, T], BF16, tag=f"kT{p}", name=f"kT{p}") for p in range(PAIRS)]
            qT = [persist.tile([128, T], BF16, tag=f"qT{p}", name=f"qT{p}") for p in range(PAIRS)]
            otu = persist.tile([65, HPC, T], BF16, tag="otu")
            otr = [persist.tile([128, T], BF16, tag=f"otr{p}", name=f"otr{p}") for p in range(PAIRS)]

            # ---- phase 1: x load + transpose (PE) ----
            with tc.tile_pool(name="xT", bufs=1) as xtp:
                xT = xtp.tile([128, KT, T], BF16, tag="xT")
                with tc.tile_pool(name="xrow", bufs=3) as xrp:
                    for ti in range(8):
                        xrow = xrp.tile([128, C], F32)
                        nc.sync.dma_start(
                            out=xrow[:], in_=x_d[ti * 128 : (ti + 1) * 128, :]
                        )
                        for g in range(2):  # ci groups of 4
                            ps = pa([128, 512], "ps_tr")
                            for j in range(4):
                                ci = g * 4 + j
                                nc.tensor.transpose(
                                    ps[:, j * 128 : (j + 1) * 128],
                                    xrow[:, ci * 128 : (ci + 1) * 128],
                                    id_t[:],
                                )
                            dst = xT[:, g * 4 : g * 4 + 4, ti * 128 : (ti + 1) * 128]
                            src = ps[:].rearrange("p (c t) -> p c t", c=4)
                            if (ti + g) % 2 == 0:
                                nc.vector.tensor_copy(dst, src)
                            else:
                                nc.scalar.copy(dst, src)

                # ---- phase 2: down-projections ----
                with tc.tile_pool(name="wr", bufs=10) as wrp, tc.tile_pool(
                    name="kvstage", bufs=2
                ) as kvs:
                    for w_d, lat, out_d in (
                        (wq_d, qlat_t, None),
                        (wk_d, klat_t, klat_d),
                        (wv_d, vlat_t, vlat_d),
                    ):
                        wr = []
                        for ci in range(8):
                            wt = wrp.tile([128, 512], BF16, tag="wt", name="wt")
                            nc.scalar.dma_start(
                                out=wt[:], in_=w_d[ci * 128 : (ci + 1) * 128, :]
                            )
                            wr.append(wt)
                        for lt in range(PAIRS):
                            ps = pa([128, T], "ps_dn")
                            for h2 in range(2):
                                for ci in range(8):
                                    nc.tensor.matmul(
                                        ps[:, h2 * 512 : (h2 + 1) * 512],
                                        wr[ci][:, lt * 128 : (lt + 1) * 128],
                                        xT[:, ci, h2 * 512 : (h2 + 1) * 512],
                                        start=(ci == 0),
                                        stop=(ci == 7),
                                    )
                            nc.vector.tensor_copy(lat[:, lt, :], ps[:])
                            if out_d is not None:
                                stage = kvs.tile(
                                    [128, T], F32, tag="kvstage", name="kvstage"
                                )
                                nc.scalar.copy(stage[:], ps[:])
                                nc.sync.dma_start(
                                    out=out_d[lt * 128 : (lt + 1) * 128, :],
                                    in_=stage[:],
                                )

            # ---- phase 3: v up-projection ----
            nc.gpsimd.memset(v_g[:, :, :, 64:65], 1.0)
            for tci in range(KT):
                ps = pa([128, 512], "ps_v")
                for p in range(PAIRS):
                    nc.tensor.matmul(
                        ps[:, p * 128 : (p + 1) * 128],
                        vlat_t[:, p, tci * 128 : (tci + 1) * 128],
                        bd_v_t[:],
                        start=True,
                        stop=True,
                    )
                nc.scalar.copy(
                    v_g[:, tci, :, 0:64],
                    ps[:].rearrange("p (h d) -> p h d", d=64),
                )

            # ---- phase 4: q/k up-projection + rope ----
            with tc.tile_pool(name="ropetmp", bufs=2) as rtp:
                for pr in range(PAIRS):
                    for bd_ce, bd_rot, lat, dst in (
                        (bd_ce_k, bd_rot_k, klat_t, kT[pr]),
                        (bd_ce_q, bd_rot_q, qlat_t, qT[pr]),
                    ):
                        ps_ce = pa([128, T], "ps_ce")
                        ps_rot = pa([128, T], "ps_rot")
                        for h2 in range(2):
                            sl = slice(h2 * 512, (h2 + 1) * 512)
                            nc.tensor.matmul(
                                ps_ce[:, sl], bd_ce[:], lat[:, pr, sl],
                                start=True, stop=True,
                            )
                            nc.tensor.matmul(
                                ps_rot[:, sl], bd_rot[:], lat[:, pr, sl],
                                start=True, stop=True,
                            )
                        nc.scalar.copy(dst[:], ps_ce[:])  # cast bf16
                        eng = nc.gpsimd if dst is qT[pr] else nc.vector
                        eng.tensor_mul(dst[:], dst[:], cc_t[:])
                        tmp = rtp.tile([128, T], BF16, tag="ropetmp", name="ropetmp")
                        nc.vector.tensor_mul(tmp[:], ps_rot[:], ss_t[:])
                        eng.tensor_add(dst[:], dst[:], tmp[:])

            # ---- phase 5: attention (+ per-pair normalization) ----
            with tc.tile_pool(name="pt", bufs=3) as ptp, tc.tile_pool(
                name="rbp", bufs=3
            ) as rbp:
                for pr in range(PAIRS):
                    ps_o = psO.tile([65, 2 * T], F32, tag="ps_o", name="ps_o")
                    for kt in range(KT):
                        koff = kt * 128
                        for qc in range(QC):
                            qlo = max(qc * 512, koff)
                            qhi = (qc + 1) * 512
                            if qhi <= koff:
                                continue
                            ncols = qhi - qlo
                            diag = qlo == koff
                            ps_s = pa([128, 2, 512], "ps_s")
                            for h2 in range(2):
                                dsl = slice(h2 * 64, (h2 + 1) * 64)
                                nc.tensor.matmul(
                                    ps_s[:, h2, 0:ncols],
                                    kT[pr][dsl, koff : koff + 128],
                                    qT[pr][dsl, qlo:qhi],
                                    start=True,
                                    stop=not diag,
                                )
                                if diag:  # add -240 above the diagonal via PE
                                    nc.tensor.matmul(
                                        ps_s[:, h2, 0:128],
                                        tmask_t[:],
                                        idbf_t[:],
                                        start=False,
                                        stop=True,
                                        skip_group_check=True,
                                    )
                            pt = ptp.tile([128, 2, 512], BF16, tag="pt", name="pt")
                            nc.scalar.activation(
                                pt[:, :, 0:ncols],
                                ps_s[:, :, 0:ncols],
                                mybir.ActivationFunctionType.Exp,
                                scale=0.125,
                            )
                            stop_kt = qc * 4 + 3
                            for h2 in range(2):
                                nc.tensor.matmul(
                                    ps_o[:, h2 * T + qlo : h2 * T + qhi],
                                    v_g[:, kt, pr * 2 + h2, :],
                                    pt[:, h2, 0:ncols],
                                    start=(kt == 0),
                                    stop=(kt == stop_kt),
                                )
                    nc.scalar.copy(
                        otu[:, pr * 2 : pr * 2 + 2, :],
                        ps_o[:].rearrange("p (h t) -> p h t", h=2),
                    )
                    # normalization for this pair, overlapped with next pair
                    r8b = rbp.tile([2, T], BF16, tag="r8b", name="r8b")
                    nc.sync.dma_start(
                        out=r8b[:],
                        in_=otu[64:65, pr * 2 : pr * 2 + 2, :],
                    )
                    r8f = rbp.tile([2, T], F32, tag="r8f", name="r8f")
                    nc.vector.tensor_copy(r8f[:], r8b[:])
                    nc.vector.reciprocal_approx_fast(out=r8f[:], in_=r8f[:])
                    r8c = rbp.tile([2, T], BF16, tag="r8c", name="r8c")
                    nc.vector.tensor_copy(r8c[:], r8f[:])
                    for h2 in range(2):
                        rsrc = r8c[h2 : h2 + 1, :]
                        src_b = bass.AP(
                            tensor=rsrc.tensor,
                            offset=rsrc.offset,
                            ap=[list(rsrc.ap[0]), [0, 64], list(rsrc.ap[1])],
                        )
                        rb = rbp.tile([64, T], BF16, tag="rb", name="rb")
                        nc.sync.dma_start(out=rb[:], in_=src_b)
                        nc.vector.tensor_mul(
                            otr[pr][h2 * 64 : (h2 + 1) * 64, :],
                            otu[0:64, pr * 2 + h2, :],
                            rb[:],
                        )

            # ---- phase 6: c_proj ----
            with tc.tile_pool(name="yout", bufs=3) as yop:
                for qt in range(8):
                    ps = pa([128, C], "ps_y")
                    for fp in range(PAIRS):
                        for half in range(2):
                            nc.tensor.matmul(
                                ps[:, half * 512 : (half + 1) * 512],
                                otr[fp][:, qt * 128 : (qt + 1) * 128],
                                wcr[fp][:, half * 512 : (half + 1) * 512],
                                start=(fp == 0),
                                stop=(fp == PAIRS - 1),
                            )
                    ysb = yop.tile([128, C], F32, tag="ysb", name="ysb")
                    nc.vector.tensor_copy(ysb[:], ps[:])
                    nc.sync.dma_start(
                        out=y_d[qt * 128 : (qt + 1) * 128, :], in_=ysb[:]
                    )

    nc.finalize()
    return nc


@functools.lru_cache(maxsize=1)
def _get_nc():
    return _build_nc()


@functools.lru_cache(maxsize=1)
def _host_tables():
    cos, sin = _rope_tables()
    cc = np.ones((128, T), dtype=np.float32)
    cc[32:64] = cos
    cc[96:128] = cos
    ss = np.zeros((128, T), dtype=np.float32)
    ss[32:64] = sin
    ss[96:128] = sin
    tri = np.tril(np.ones((128, 128), dtype=np.float32)).T  # [k, q]: 1 if k<=q
    tri2 = np.stack([tri, tri], axis=1)  # [128, 2, 128]
    ident = np.eye(128, dtype=np.float32)
    return cc, ss, tri2, ident


def _bf16(a):
    import ml_dtypes

    return np.asarray(a, dtype=ml_dtypes.bfloat16)


def kernel(x, Wq_down, Wk_down, Wv_down, Wq_up_c, Wq_up_e, Wk_up_c, Wk_up_e, Wv_up, Wc):
    x = np.asarray(x, dtype=np.float32)
    Wq_down = np.asarray(Wq_down, dtype=np.float32)
    Wk_down = np.asarray(Wk_down, dtype=np.float32)
    Wv_down = np.asarray(Wv_down, dtype=np.float32)
    Wc = np.asarray(Wc, dtype=np.float32)

    cc, ss, tri2, ident = _host_tables()
    tmask = np.triu(np.full((128, 128), -240.0, dtype=np.float32), 1)
    bd_ce_q = _blockdiag2(
        np.concatenate([np.asarray(Wq_up_c), np.asarray(Wq_up_e)], axis=1)
    ).astype(np.float32)
    bd_ce_k = _blockdiag2(
        np.concatenate([np.asarray(Wk_up_c), np.asarray(Wk_up_e)], axis=1)
    ).astype(np.float32)
    bd_rot_q = _rot_ext(np.asarray(Wq_up_e))
    bd_rot_k = _rot_ext(np.asarray(Wk_up_e))
    bd_v = _bf16(_blockdiag2(np.asarray(Wv_up)))

    nc = _get_nc()
    in_maps = []
    for core in range(NCORES):
        b, hh = core // 2, core % 2
        sl = slice(hh * 512, (hh + 1) * 512)
        in_maps.append(
            {
                "x": x[b],
                "wq": _bf16(Wq_down[:, sl]),
                "wk": _bf16(Wk_down[:, sl]),
                "wv": _bf16(Wv_down[:, sl]),
                "wc": _bf16(Wc[sl, :]),
                "bd_ce_q": _bf16(bd_ce_q),
                "bd_ce_k": _bf16(bd_ce_k),
                "bd_rot_q": _bf16(bd_rot_q),
                "bd_rot_k": _bf16(bd_rot_k),
                "bd_v": bd_v,
                "cc": _bf16(cc),
                "ss": _bf16(ss),
                "tri2": _bf16(tri2),
                "tmask": _bf16(tmask),
                "idbf": _bf16(ident),
                "ident": ident,
            }
        )

    kwargs = {}
    if TRACE:
        import shutil, os
        shutil.rmtree("/tmp/mla_trace", ignore_errors=True)
        os.makedirs("/tmp/mla_trace", exist_ok=True)
        kwargs = dict(trace=True, tmpdir="/tmp/mla_trace")
    res = run_bass_kernel_spmd(nc, in_maps, core_ids=list(range(NCORES)), **kwargs)
    if TRACE:
        kernel.last_exec_time_ns = res.exec_time_ns

    y = np.empty((B, T, C), dtype=np.float32)
    k_lat = np.empty((B, T, H, L), dtype=np.float32)
    v_lat = np.empty((B, T, H, L), dtype=np.float32)
    for b in range(B):
        r0, r1 = res.results[2 * b], res.results[2 * b + 1]
        y[b] = r0["y_part"] + r1["y_part"]
        for hh, r in ((0, r0), (1, r1)):
            hs = slice(hh * 8, hh * 8 + 8)
            k_lat[b, :, hs, :] = r["klatT"].reshape(8, 64, T).transpose(2, 0, 1)
            v_lat[b, :, hs, :] = r["vlatT"].reshape(8, 64, T).transpose(2, 0, 1)
    return y, k_lat, v_lat


# revision 34
# speedup vs baseline: 1.0444x; 1.0444x over previous
"""Trainium2 Bass kernel for MLA self-attention (nn_MLASelfAttentionWithMoBA).

Sharding: 8 cores = 4 batches x 2 head-halves (tensor parallel over heads).
Each core handles one batch element and 8 of the 16 heads:
  - down-projections x @ W*_down restricted to its heads' latent columns
  - per-head up-projections + partial RoPE (pair-mixing folded into a
    host-precomputed "rotated" copy of the up-projection weights)
  - full causal attention for its 8 heads, scores kept transposed [k, q]
    so A@V needs no P transposes; softmax denominators come from an extra
    ones-column appended to V; normalization is applied to the attention
    output before the c_proj
  - row-parallel c_proj with its 512 rows of Wc -> partial y
Host combines: y = y_half0 + y_half1 per batch; k_lat/v_lat are written
head-major-transposed [512, T] per core and transposed on the host.

Down-projection runs in f32r (tf32-like, full PE rate) so the k_lat/v_lat
outputs keep ~1e-4 precision; attention/up/c_proj operands are bf16/f32r.
"""

import functools

import numpy as np

import concourse.bass as bass
import concourse.tile as tile
from concourse import bacc, mybir
from concourse.bass_utils import run_bass_kernel_spmd

F32 = mybir.dt.float32
F32R = mybir.dt.float32r
BF16 = mybir.dt.bfloat16

B, T, C, H, L = 4, 1024, 1024, 16, 64
DH, DE = 64, 32
NCORES = 8
HPC = H // 2  # heads per core (8)
PAIRS = HPC // 2  # head pairs per core (4)
KT = T // 128  # key tiles (8)
QC = T // 512  # q chunks (2)

TRACE = False  # set by test.py to capture an NTFF profile


def _rope_tables():
    inv = 1.0 / (10000.0 ** (np.arange(0, DE, 2, dtype=np.float64) / DE))  # [16]
    t = np.arange(T, dtype=np.float64)
    freqs = np.outer(t, inv)  # [T, 16]
    emb = np.concatenate([freqs, freqs], axis=-1)  # [T, 32]
    cos = np.cos(emb).T.astype(np.float32)  # [32, T]
    sin = np.sin(emb).T.astype(np.float32)
    return cos, sin


def _rot_cols(w):
    """w: [L, DE]. Columns permuted/negated so lat@w_rot == rotate_half(lat@w)."""
    r = np.empty_like(w)
    r[:, 0::2] = -w[:, 1::2]
    r[:, 1::2] = w[:, 0::2]
    return r


def _blockdiag2(a):
    z = np.zeros_like(a)
    return np.block([[a, z], [z, a]])  # [2m, 2n]


def _rot_ext(w):
    """[128, 128] table: out rows 32-63 <- rot-proj(h0), rows 96-127 <- rot-proj(h1),
    other out rows 0 (so the rope add is one full-height op)."""
    r = _rot_cols(w)  # [64, 32]
    ext = np.zeros((128, 128), dtype=np.float32)
    ext[0:64, 32:64] = r
    ext[64:128, 96:128] = r
    return ext


def _build_nc():
    nc = bacc.Bacc("TRN2", target_bir_lowering=False, debug=False, num_devices=NCORES)

    x_d = nc.dram_tensor("x", [T, C], F32, kind="ExternalInput")
    wq_d = nc.dram_tensor("wq", [C, 512], BF16, kind="ExternalInput")
    wk_d = nc.dram_tensor("wk", [C, 512], BF16, kind="ExternalInput")
    wv_d = nc.dram_tensor("wv", [C, 512], BF16, kind="ExternalInput")
    wc_d = nc.dram_tensor("wc", [512, C], BF16, kind="ExternalInput")
    bd_ce_q_d = nc.dram_tensor("bd_ce_q", [128, 128], BF16, kind="ExternalInput")
    bd_ce_k_d = nc.dram_tensor("bd_ce_k", [128, 128], BF16, kind="ExternalInput")
    bd_rot_q_d = nc.dram_tensor("bd_rot_q", [128, 128], BF16, kind="ExternalInput")
    bd_rot_k_d = nc.dram_tensor("bd_rot_k", [128, 128], BF16, kind="ExternalInput")
    bd_v_d = nc.dram_tensor("bd_v", [128, 128], BF16, kind="ExternalInput")
    cc_d = nc.dram_tensor("cc", [128, T], BF16, kind="ExternalInput")
    ss_d = nc.dram_tensor("ss", [128, T], BF16, kind="ExternalInput")
    tri2_d = nc.dram_tensor("tri2", [128, 2, 128], BF16, kind="ExternalInput")
    tmask_d = nc.dram_tensor("tmask", [128, 128], BF16, kind="ExternalInput")
    idbf_d = nc.dram_tensor("idbf", [128, 128], BF16, kind="ExternalInput")
    ident_d = nc.dram_tensor("ident", [128, 128], F32, kind="ExternalInput")

    y_d = nc.dram_tensor("y_part", [T, C], F32, kind="ExternalOutput")
    klat_d = nc.dram_tensor("klatT", [512, T], F32, kind="ExternalOutput")
    vlat_d = nc.dram_tensor("vlatT", [512, T], F32, kind="ExternalOutput")

    with tile.TileContext(nc) as tc:
        with tc.tile_pool(name="const", bufs=1) as const, tc.tile_pool(
            name="persist", bufs=1
        ) as persist, tc.tile_pool(
            name="psA", bufs=2, space="PSUM"
        ) as psA, tc.tile_pool(name="psO", bufs=1, space="PSUM") as psO:

            def pa(shape, name):
                padded = [128] + list(shape[1:-1]) + [T // (int(np.prod(shape[1:-1])) or 1)]
                return psA.tile(shape, F32, tag="A", name=name, padded_shape=padded)

            # ---- constants ----
            cc_t = const.tile([128, T], BF16, tag="cc")
            nc.sync.dma_start(out=cc_t[:], in_=cc_d[:])
            ss_t = const.tile([128, T], BF16, tag="ss")
            nc.sync.dma_start(out=ss_t[:], in_=ss_d[:])
            tri2_t = const.tile([128, 2, 128], BF16, tag="tri2")
            nc.sync.dma_start(out=tri2_t[:], in_=tri2_d[:])
            id_t = const.tile([128, 128], F32, tag="ident")
            nc.sync.dma_start(out=id_t[:], in_=ident_d[:])
            bd_ce_q = const.tile([128, 128], BF16, tag="bd_ce_q")
            nc.sync.dma_start(out=bd_ce_q[:], in_=bd_ce_q_d[:])
            bd_ce_k = const.tile([128, 128], BF16, tag="bd_ce_k")
            nc.sync.dma_start(out=bd_ce_k[:], in_=bd_ce_k_d[:])
            bd_rot_q = const.tile([128, 128], BF16, tag="bd_rot_q")
            nc.sync.dma_start(out=bd_rot_q[:], in_=bd_rot_q_d[:])
            bd_rot_k = const.tile([128, 128], BF16, tag="bd_rot_k")
            nc.sync.dma_start(out=bd_rot_k[:], in_=bd_rot_k_d[:])
            bd_v_t = const.tile([128, 128], BF16, tag="bd_v")
            nc.sync.dma_start(out=bd_v_t[:], in_=bd_v_d[:])
            tmask_t = const.tile([128, 128], BF16, tag="tmask")
            nc.sync.dma_start(out=tmask_t[:], in_=tmask_d[:])
            idbf_t = const.tile([128, 128], BF16, tag="idbf")
            nc.sync.dma_start(out=idbf_t[:], in_=idbf_d[:])

            # PE warmup burst so the HAM clock-gate reaches K=8 early
            wsrc = const.tile([128, 512], BF16, tag="wsrc")
            nc.vector.memset(wsrc[:], 1.0)
            for wi in range(8):
                pw = pa([128, 512], "pw")
                nc.tensor.matmul(pw[:], wsrc[:, 0:128], wsrc[:], start=True, stop=True)

            # Wc: load early so c_proj never waits on it
            wcr = []
            for fp in range(PAIRS):
                wt = persist.tile([128, C], BF16, tag=f"wcr{fp}", name=f"wcr{fp}")
                nc.scalar.dma_start(out=wt[:], in_=wc_d[fp * 128 : (fp + 1) * 128, :])
                wcr.append(wt)

            # persistent intermediates (bf16 to fit SBUF)
            qlat_t = persist.tile([128, PAIRS, T], BF16, tag="qlat")
            klat_t = persist.tile([128, PAIRS, T], BF16, tag="klat")
            vlat_t = persist.tile([128, PAIRS, T], BF16, tag="vlat")
            v_g = persist.tile([128, KT, HPC, 65], BF16, tag="v_g")
            kT = [persist.tile([128, T], BF16, tag=f"kT{p}", name=f"kT{p}") for p in range(PAIRS)]
            qT = [persist.tile([128, T], BF16, tag=f"qT{p}", name=f"qT{p}") for p in range(PAIRS)]
            otu = persist.tile([65, HPC, T], BF16, tag="otu")
            otr = [persist.tile([128, T], BF16, tag=f"otr{p}", name=f"otr{p}") for p in range(PAIRS)]

            # ---- phase 1: x load + transpose (PE) ----
            with tc.tile_pool(name="xT", bufs=1) as xtp:
                xT = xtp.tile([128, KT, T], BF16, tag="xT")
                with tc.tile_pool(name="xrow", bufs=3) as xrp:
                    for ti in range(8):
                        xrow = xrp.tile([128, C], F32)
                        nc.sync.dma_start(
                            out=xrow[:], in_=x_d[ti * 128 : (ti + 1) * 128, :]
                        )
                        for g in range(2):  # ci groups of 4
                            ps = pa([128, 512], "ps_tr")
                            for j in range(4):
                                ci = g * 4 + j
                                nc.tensor.transpose(
                                    ps[:, j * 128 : (j + 1) * 128],
                                    xrow[:, ci * 128 : (ci + 1) * 128],
                                    id_t[:],
                                )
                            dst = xT[:, g * 4 : g * 4 + 4, ti * 128 : (ti + 1) * 128]
                            src = ps[:].rearrange("p (c t) -> p c t", c=4)
                            if (ti + g) % 2 == 0:
                                nc.vector.tensor_copy(dst, src)
                            else:
                                nc.scalar.copy(dst, src)

                # ---- phase 2: down-projections ----
                with tc.tile_pool(name="wr", bufs=10) as wrp, tc.tile_pool(
                    name="kvstage", bufs=2
                ) as kvs:
                    for w_d, lat, out_d in (
                        (wq_d, qlat_t, None),
                        (wk_d, klat_t, klat_d),
                        (wv_d, vlat_t, vlat_d),
                    ):
                        wr = []
                        for ci in range(8):
                            wt = wrp.tile([128, 512], BF16, tag="wt", name="wt")
                            nc.scalar.dma_start(
                                out=wt[:], in_=w_d[ci * 128 : (ci + 1) * 128, :]
                            )
                            wr.append(wt)
                        for lt in range(PAIRS):
                            ps = pa([128, T], "ps_dn")
                            for h2 in range(2):
                                for ci in range(8):
                                    nc.tensor.matmul(
                                        ps[:, h2 * 512 : (h2 + 1) * 512],
                                        wr[ci][:, lt * 128 : (lt + 1) * 128],
                                        xT[:, ci, h2 * 512 : (h2 + 1) * 512],
                                        start=(ci == 0),
                                        stop=(ci == 7),
                                    )
                            nc.vector.tensor_copy(lat[:, lt, :], ps[:])
                            if out_d is not None:
                                stage = kvs.tile(
                                    [128, T], F32, tag="kvstage", name="kvstage"
                                )
                                nc.scalar.copy(stage[:], ps[:])
                                nc.sync.dma_start(
                                    out=out_d[lt * 128 : (lt + 1) * 128, :],
                                    in_=stage[:],
                                )

            # ---- phase 3: v up-projection ----
            nc.gpsimd.memset(v_g[:, :, :, 64:65], 1.0)
            for tci in range(KT):
                ps = pa([128, 512], "ps_v")
                for p in range(PAIRS):
                    nc.tensor.matmul(
                        ps[:, p * 128 : (p + 1) * 128],
                        vlat_t[:, p, tci * 128 : (tci + 1) * 128],
                        bd_v_t[:],
                        start=True,
                        stop=True,
                    )
                nc.scalar.copy(
                    v_g[:, tci, :, 0:64],
                    ps[:].rearrange("p (h d) -> p h d", d=64),
                )

            # ---- phase 4: q/k up-projection + rope ----
            with tc.tile_pool(name="ropetmp", bufs=2) as rtp:
                for pr in range(PAIRS):
                    for bd_ce, bd_rot, lat, dst in (
                        (bd_ce_k, bd_rot_k, klat_t, kT[pr]),
                        (bd_ce_q, bd_rot_q, qlat_t, qT[pr]),
                    ):
                        ps_ce = pa([128, T], "ps_ce")
                        ps_rot = pa([128, T], "ps_rot")
                        for h2 in range(2):
                            sl = slice(h2 * 512, (h2 + 1) * 512)
                            nc.tensor.matmul(
                                ps_ce[:, sl], bd_ce[:], lat[:, pr, sl],
                                start=True, stop=True,
                            )
                            nc.tensor.matmul(
                                ps_rot[:, sl], bd_rot[:], lat[:, pr, sl],
                                start=True, stop=True,
                            )
                        nc.scalar.copy(dst[:], ps_ce[:])  # cast bf16
                        eng = nc.gpsimd if dst is qT[pr] else nc.vector
                        eng.tensor_mul(dst[:], dst[:], cc_t[:])
                        tmp = rtp.tile([128, T], BF16, tag="ropetmp", name="ropetmp")
                        nc.vector.tensor_mul(tmp[:], ps_rot[:], ss_t[:])
                        eng.tensor_add(dst[:], dst[:], tmp[:])

            # ---- phase 5: attention (+ per-pair normalization) ----
            with tc.tile_pool(name="pt", bufs=3) as ptp, tc.tile_pool(
                name="rbp", bufs=3
            ) as rbp:
                for pr in range(PAIRS):
                    ps_o = psO.tile([65, 2 * T], F32, tag="ps_o", name="ps_o")
                    for kt in range(KT):
                        koff = kt * 128
                        for qc in range(QC):
                            qlo = max(qc * 512, koff)
                            qhi = (qc + 1) * 512
                            if qhi <= koff:
                                continue
                            ncols = qhi - qlo
                            ps_s = pa([128, 2, 512], "ps_s")
                            for h2 in range(2):
                                dsl = slice(h2 * 64, (h2 + 1) * 64)
                                nc.tensor.matmul(
                                    ps_s[:, h2, 0:ncols],
                                    kT[pr][dsl, koff : koff + 128],
                                    qT[pr][dsl, qlo:qhi],
                                    start=True,
                                    stop=True,
                                )
                            pt = ptp.tile([128, 2, 512], BF16, tag="pt", name="pt")
                            nc.scalar.activation(
                                pt[:, :, 0:ncols],
                                ps_s[:, :, 0:ncols],
                                mybir.ActivationFunctionType.Exp,
                                scale=0.125,
                            )
                            if qlo == koff:  # diagonal 128-block lives here
                                nc.gpsimd.tensor_mul(
                                    pt[:, :, 0:128], pt[:, :, 0:128], tri2_t[:]
                                )
                            stop_kt = qc * 4 + 3
                            for h2 in range(2):
                                nc.tensor.matmul(
                                    ps_o[:, h2 * T + qlo : h2 * T + qhi],
                                    v_g[:, kt, pr * 2 + h2, :],
                                    pt[:, h2, 0:ncols],
                                    start=(kt == 0),
                                    stop=(kt == stop_kt),
                                )
                    nc.scalar.copy(
                        otu[:, pr * 2 : pr * 2 + 2, :],
                        ps_o[:].rearrange("p (h t) -> p h t", h=2),
                    )
                    # normalization for this pair, overlapped with next pair
                    r8b = rbp.tile([2, T], BF16, tag="r8b", name="r8b")
                    nc.sync.dma_start(
                        out=r8b[:],
                        in_=otu[64:65, pr * 2 : pr * 2 + 2, :],
                    )
                    r8f = rbp.tile([2, T], F32, tag="r8f", name="r8f")
                    nc.vector.tensor_copy(r8f[:], r8b[:])
                    nc.vector.reciprocal_approx_fast(out=r8f[:], in_=r8f[:])
                    r8c = rbp.tile([2, T], BF16, tag="r8c", name="r8c")
                    nc.vector.tensor_copy(r8c[:], r8f[:])
                    for h2 in range(2):
                        rsrc = r8c[h2 : h2 + 1, :]
                        src_b = bass.AP(
                            tensor=rsrc.tensor,
                            offset=rsrc.offset,
                            ap=[list(rsrc.ap[0]), [0, 64], list(rsrc.ap[1])],
                        )
                        rb = rbp.tile([64, T], BF16, tag="rb", name="rb")
                        nc.sync.dma_start(out=rb[:], in_=src_b)
                        nc.vector.tensor_mul(
                            otr[pr][h2 * 64 : (h2 + 1) * 64, :],
                            otu[0:64, pr * 2 + h2, :],
                            rb[:],
                        )

            # ---- phase 6: c_proj ----
            with tc.tile_pool(name="yout", bufs=3) as yop:
                for qt in range(8):
                    ps = pa([128, C], "ps_y")
                    for fp in range(PAIRS):
                        for half in range(2):
                            nc.tensor.matmul(
                                ps[:, half * 512 : (half + 1) * 512],
                                otr[fp][:, qt * 128 : (qt + 1) * 128],
                                wcr[fp][:, half * 512 : (half + 1) * 512],
                                start=(fp == 0),
                                stop=(fp == PAIRS - 1),
                            )
                    ysb = yop.tile([128, C], F32, tag="ysb", name="ysb")
                    nc.vector.tensor_copy(ysb[:], ps[:])
                    nc.sync.dma_start(
                        out=y_d[qt * 128 : (qt + 1) * 128, :], in_=ysb[:]
                    )

    nc.finalize()
    return nc


@functools.lru_cache(maxsize=1)
def _get_nc():
    return _build_nc()


@functools.lru_cache(maxsize=1)
def _host_tables():
    cos, sin = _rope_tables()
    cc = np.ones((128, T), dtype=np.float32)
    cc[32:64] = cos
    cc[96:128] = cos
    ss = np.zeros((128, T), dtype=np.float32)
    ss[32:64] = sin
    ss[96:128] = sin
    tri = np.tril(np.ones((128, 128), dtype=np.float32)).T  # [k, q]: 1 if k<=q
    tri2 = np.stack([tri, tri], axis=1)  # [128, 2, 128]
    ident = np.eye(128, dtype=np.float32)
    return cc, ss, tri2, ident


def _bf16(a):
    import ml_dtypes

    return np.asarray(a, dtype=ml_dtypes.bfloat16)


def kernel(x, Wq_down, Wk_down, Wv_down, Wq_up_c, Wq_up_e, Wk_up_c, Wk_up_e, Wv_up, Wc):
    x = np.asarray(x, dtype=np.float32)
    Wq_down = np.asarray(Wq_down, dtype=np.float32)
    Wk_down = np.asarray(Wk_down, dtype=np.float32)
    Wv_down = np.asarray(Wv_down, dtype=np.float32)
    Wc = np.asarray(Wc, dtype=np.float32)

    cc, ss, tri2, ident = _host_tables()
    tmask = np.triu(np.full((128, 128), -240.0, dtype=np.float32), 1)
    bd_ce_q = _blockdiag2(
        np.concatenate([np.asarray(Wq_up_c), np.asarray(Wq_up_e)], axis=1)
    ).astype(np.float32)
    bd_ce_k = _blockdiag2(
        np.concatenate([np.asarray(Wk_up_c), np.asarray(Wk_up_e)], axis=1)
    ).astype(np.float32)
    bd_rot_q = _rot_ext(np.asarray(Wq_up_e))
    bd_rot_k = _rot_ext(np.asarray(Wk_up_e))
    bd_v = _bf16(_blockdiag2(np.asarray(Wv_up)))

    nc = _get_nc()
    in_maps = []
    for core in range(NCORES):
        b, hh = core // 2, core % 2
        sl = slice(hh * 512, (hh + 1) * 512)
        in_maps.append(
            {
                "x": x[b],
                "wq": _bf16(Wq_down[:, sl]),
                "wk": _bf16(Wk_down[:, sl]),
                "wv": _bf16(Wv_down[:, sl]),
                "wc": _bf16(Wc[sl, :]),
                "bd_ce_q": _bf16(bd_ce_q),
                "bd_ce_k": _bf16(bd_ce_k),
                "bd_rot_q": _bf16(bd_rot_q),
                "bd_rot_k": _bf16(bd_rot_k),
                "bd_v": bd_v,
                "cc": _bf16(cc),
                "ss": _bf16(ss),
                "tri2": _bf16(tri2),
                "tmask": _bf16(tmask),
                "idbf": _bf16(ident),
                "ident": ident,
            }
        )

    kwargs = {}
    if TRACE:
        import shutil, os
        shutil.rmtree("/tmp/mla_trace", ignore_errors=True)
        os.makedirs("/tmp/mla_trace", exist_ok=True)
        kwargs = dict(trace=True, tmpdir="/tmp/mla_trace")
    res = run_bass_kernel_spmd(nc, in_maps, core_ids=list(range(NCORES)), **kwargs)
    if TRACE:
        kernel.last_exec_time_ns = res.exec_time_ns

    y = np.empty((B, T, C), dtype=np.float32)
    k_lat = np.empty((B, T, H, L), dtype=np.float32)
    v_lat = np.empty((B, T, H, L), dtype=np.float32)
    for b in range(B):
        r0, r1 = res.results[2 * b], res.results[2 * b + 1]
        y[b] = r0["y_part"] + r1["y_part"]
        for hh, r in ((0, r0), (1, r1)):
            hs = slice(hh * 8, hh * 8 + 8)
            k_lat[b, :, hs, :] = r["klatT"].reshape(8, 64, T).transpose(2, 0, 1)
            v_lat[b, :, hs, :] = r["vlatT"].reshape(8, 64, T).transpose(2, 0, 1)
    return y, k_lat, v_lat


# revision 35
# speedup vs baseline: 1.0772x; 1.0313x over previous
"""Trainium2 Bass kernel for MLA self-attention (nn_MLASelfAttentionWithMoBA).

Sharding: 8 cores = 4 batches x 2 head-halves (tensor parallel over heads).
Each core handles one batch element and 8 of the 16 heads:
  - down-projections x @ W*_down restricted to its heads' latent columns
  - per-head up-projections + partial RoPE (pair-mixing folded into a
    host-precomputed "rotated" copy of the up-projection weights)
  - full causal attention for its 8 heads, scores kept transposed [k, q]
    so A@V needs no P transposes; softmax denominators come from an extra
    ones-column appended to V; normalization is applied to the attention
    output before the c_proj
  - row-parallel c_proj with its 512 rows of Wc -> partial y
Host combines: y = y_half0 + y_half1 per batch; k_lat/v_lat are written
head-major-transposed [512, T] per core and transposed on the host.

Down-projection runs in f32r (tf32-like, full PE rate) so the k_lat/v_lat
outputs keep ~1e-4 precision; attention/up/c_proj operands are bf16/f32r.
"""

import functools

import numpy as np

import concourse.bass as bass
import concourse.tile as tile
from concourse import bacc, mybir
from concourse.bass_utils import run_bass_kernel_spmd

F32 = mybir.dt.float32
F32R = mybir.dt.float32r
BF16 = mybir.dt.bfloat16

B, T, C, H, L = 4, 1024, 1024, 16, 64
DH, DE = 64, 32
NCORES = 8
HPC = H // 2  # heads per core (8)
PAIRS = HPC // 2  # head pairs per core (4)
KT = T // 128  # key tiles (8)
QC = T // 512  # q chunks (2)

TRACE = False  # set by test.py to capture an NTFF profile


def _rope_tables():
    inv = 1.0 / (10000.0 ** (np.arange(0, DE, 2, dtype=np.float64) / DE))  # [16]
    t = np.arange(T, dtype=np.float64)
    freqs = np.outer(t, inv)  # [T, 16]
    emb = np.concatenate([freqs, freqs], axis=-1)  # [T, 32]
    cos = np.cos(emb).T.astype(np.float32)  # [32, T]
    sin = np.sin(emb).T.astype(np.float32)
    return cos, sin


def _rot_cols(w):
    """w: [L, DE]. Columns permuted/negated so lat@w_rot == rotate_half(lat@w)."""
    r = np.empty_like(w)
    r[:, 0::2] = -w[:, 1::2]
    r[:, 1::2] = w[:, 0::2]
    return r


def _blockdiag2(a):
    z = np.zeros_like(a)
    return np.block([[a, z], [z, a]])  # [2m, 2n]


def _rot_ext(w):
    """[128, 128] table: out rows 32-63 <- rot-proj(h0), rows 96-127 <- rot-proj(h1),
    other out rows 0 (so the rope add is one full-height op)."""
    r = _rot_cols(w)  # [64, 32]
    ext = np.zeros((128, 128), dtype=np.float32)
    ext[0:64, 32:64] = r
    ext[64:128, 96:128] = r
    return ext


def _build_nc():
    nc = bacc.Bacc("TRN2", target_bir_lowering=False, debug=False, num_devices=NCORES)

    x_d = nc.dram_tensor("x", [T, C], F32, kind="ExternalInput")
    wq_d = nc.dram_tensor("wq", [C, 512], BF16, kind="ExternalInput")
    wk_d = nc.dram_tensor("wk", [C, 512], BF16, kind="ExternalInput")
    wv_d = nc.dram_tensor("wv", [C, 512], BF16, kind="ExternalInput")
    wc_d = nc.dram_tensor("wc", [512, C], BF16, kind="ExternalInput")
    bd_ce_q_d = nc.dram_tensor("bd_ce_q", [128, 128], BF16, kind="ExternalInput")
    bd_ce_k_d = nc.dram_tensor("bd_ce_k", [128, 128], BF16, kind="ExternalInput")
    bd_rot_q_d = nc.dram_tensor("bd_rot_q", [128, 128], BF16, kind="ExternalInput")
    bd_rot_k_d = nc.dram_tensor("bd_rot_k", [128, 128], BF16, kind="ExternalInput")
    bd_v_d = nc.dram_tensor("bd_v", [128, 128], BF16, kind="ExternalInput")
    cc_d = nc.dram_tensor("cc", [128, T], BF16, kind="ExternalInput")
    ss_d = nc.dram_tensor("ss", [128, T], BF16, kind="ExternalInput")
    tri2_d = nc.dram_tensor("tri2", [128, 2, 128], BF16, kind="ExternalInput")
    tmask_d = nc.dram_tensor("tmask", [128, 128], BF16, kind="ExternalInput")
    idbf_d = nc.dram_tensor("idbf", [128, 128], BF16, kind="ExternalInput")
    ident_d = nc.dram_tensor("ident", [128, 128], F32, kind="ExternalInput")

    y_d = nc.dram_tensor("y_part", [T, C], F32, kind="ExternalOutput")
    klat_d = nc.dram_tensor("klatT", [512, T], F32, kind="ExternalOutput")
    vlat_d = nc.dram_tensor("vlatT", [512, T], F32, kind="ExternalOutput")

    with tile.TileContext(nc) as tc:
        with tc.tile_pool(name="const", bufs=1) as const, tc.tile_pool(
            name="persist", bufs=1
        ) as persist, tc.tile_pool(
            name="psA", bufs=2, space="PSUM"
        ) as psA, tc.tile_pool(name="psO", bufs=1, space="PSUM") as psO:

            def pa(shape, name):
                padded = [128] + list(shape[1:-1]) + [T // (int(np.prod(shape[1:-1])) or 1)]
                return psA.tile(shape, F32, tag="A", name=name, padded_shape=padded)

            # ---- constants ----
            cc_t = const.tile([128, T], BF16, tag="cc")
            nc.sync.dma_start(out=cc_t[:], in_=cc_d[:])
            ss_t = const.tile([128, T], BF16, tag="ss")
            nc.sync.dma_start(out=ss_t[:], in_=ss_d[:])
            tri2_t = const.tile([128, 2, 128], BF16, tag="tri2")
            nc.sync.dma_start(out=tri2_t[:], in_=tri2_d[:])
            id_t = const.tile([128, 128], F32, tag="ident")
            nc.sync.dma_start(out=id_t[:], in_=ident_d[:])
            bd_ce_q = const.tile([128, 128], BF16, tag="bd_ce_q")
            nc.sync.dma_start(out=bd_ce_q[:], in_=bd_ce_q_d[:])
            bd_ce_k = const.tile([128, 128], BF16, tag="bd_ce_k")
            nc.sync.dma_start(out=bd_ce_k[:], in_=bd_ce_k_d[:])
            bd_rot_q = const.tile([128, 128], BF16, tag="bd_rot_q")
            nc.sync.dma_start(out=bd_rot_q[:], in_=bd_rot_q_d[:])
            bd_rot_k = const.tile([128, 128], BF16, tag="bd_rot_k")
            nc.sync.dma_start(out=bd_rot_k[:], in_=bd_rot_k_d[:])
            bd_v_t = const.tile([128, 128], BF16, tag="bd_v")
            nc.sync.dma_start(out=bd_v_t[:], in_=bd_v_d[:])
            tmask_t = const.tile([128, 128], BF16, tag="tmask")
            nc.sync.dma_start(out=tmask_t[:], in_=tmask_d[:])
            idbf_t = const.tile([128, 128], BF16, tag="idbf")
            nc.sync.dma_start(out=idbf_t[:], in_=idbf_d[:])

            # PE warmup burst so the HAM clock-gate reaches K=8 early
            wsrc = const.tile([128, 512], BF16, tag="wsrc")
            nc.vector.memset(wsrc[:], 1.0)
            for wi in range(8):
                pw = pa([128, 512], "pw")
                nc.tensor.matmul(pw[:], wsrc[:, 0:128], wsrc[:], start=True, stop=True)

            # Wc: load early so c_proj never waits on it
            wcr = []
            for fp in range(PAIRS):
                wt = persist.tile([128, C], BF16, tag=f"wcr{fp}", name=f"wcr{fp}")
                nc.scalar.dma_start(out=wt[:], in_=wc_d[fp * 128 : (fp + 1) * 128, :])
                wcr.append(wt)

            # persistent intermediates (bf16 to fit SBUF)
            qlat_t = persist.tile([128, PAIRS, T], BF16, tag="qlat")
            klat_t = persist.tile([128, PAIRS, T], BF16, tag="klat")
            vlat_t = persist.tile([128, PAIRS, T], BF16, tag="vlat")
            v_g = persist.tile([128, KT, HPC, 65], BF16, tag="v_g")
            kT = [persist.tile([128, T], BF16, tag=f"kT{p}", name=f"kT{p}") for p in range(PAIRS)]
            qT = [persist.tile([128, T], BF16, tag=f"qT{p}", name=f"qT{p}") for p in range(PAIRS)]
            otu = persist.tile([65, HPC, T], BF16, tag="otu")
            otr = [persist.tile([128, T], BF16, tag=f"otr{p}", name=f"otr{p}") for p in range(PAIRS)]

            # ---- phase 1: x load + transpose (PE) ----
            with tc.tile_pool(name="xT", bufs=1) as xtp:
                xT = xtp.tile([128, KT, T], BF16, tag="xT")
                with tc.tile_pool(name="xrow", bufs=3) as xrp:
                    for ti in range(8):
                        xrow = xrp.tile([128, C], F32)
                        nc.sync.dma_start(
                            out=xrow[:], in_=x_d[ti * 128 : (ti + 1) * 128, :]
                        )
                        for g in range(2):  # ci groups of 4
                            ps = pa([128, 512], "ps_tr")
                            for j in range(4):
                                ci = g * 4 + j
                                nc.tensor.transpose(
                                    ps[:, j * 128 : (j + 1) * 128],
                                    xrow[:, ci * 128 : (ci + 1) * 128],
                                    id_t[:],
                                )
                            dst = xT[:, g * 4 : g * 4 + 4, ti * 128 : (ti + 1) * 128]
                            src = ps[:].rearrange("p (c t) -> p c t", c=4)
                            if (ti + g) % 2 == 0:
                                nc.vector.tensor_copy(dst, src)
                            else:
                                nc.scalar.copy(dst, src)

                # ---- phase 2: down-projections ----
                with tc.tile_pool(name="wr", bufs=10) as wrp, tc.tile_pool(
                    name="kvstage", bufs=2
                ) as kvs:
                    for w_d, lat, out_d in (
                        (wq_d, qlat_t, None),
                        (wk_d, klat_t, klat_d),
                        (wv_d, vlat_t, vlat_d),
                    ):
                        wr = []
                        for ci in range(8):
                            wt = wrp.tile([128, 512], BF16, tag="wt", name="wt")
                            nc.scalar.dma_start(
                                out=wt[:], in_=w_d[ci * 128 : (ci + 1) * 128, :]
                            )
                            wr.append(wt)
                        for lt in range(PAIRS):
                            ps = pa([128, T], "ps_dn")
                            for ci in range(8):
                                for h2 in range(2):
                                    nc.tensor.matmul(
                                        ps[:, h2 * 512 : (h2 + 1) * 512],
                                        wr[ci][:, lt * 128 : (lt + 1) * 128],
                                        xT[:, ci, h2 * 512 : (h2 + 1) * 512],
                                        start=(ci == 0),
                                        stop=(ci == 7),
                                    )
                            nc.vector.tensor_copy(lat[:, lt, :], ps[:])
                            if out_d is not None:
                                stage = kvs.tile(
                                    [128, T], F32, tag="kvstage", name="kvstage"
                                )
                                nc.scalar.copy(stage[:], ps[:])
                                nc.sync.dma_start(
                                    out=out_d[lt * 128 : (lt + 1) * 128, :],
                                    in_=stage[:],
                                )

            # ---- phase 3: v up-projection ----
            nc.gpsimd.memset(v_g[:, :, :, 64:65], 1.0)
            for tci in range(KT):
                ps = pa([128, 512], "ps_v")
                for p in range(PAIRS):
                    nc.tensor.matmul(
                        ps[:, p * 128 : (p + 1) * 128],
                        vlat_t[:, p, tci * 128 : (tci + 1) * 128],
                        bd_v_t[:],
                        start=True,
                        stop=True,
                    )
                nc.scalar.copy(
                    v_g[:, tci, :, 0:64],
                    ps[:].rearrange("p (h d) -> p h d", d=64),
                )

            # ---- phase 4: q/k up-projection + rope ----
            with tc.tile_pool(name="ropetmp", bufs=2) as rtp:
                for pr in range(PAIRS):
                    for bd_ce, bd_rot, lat, dst in (
                        (bd_ce_k, bd_rot_k, klat_t, kT[pr]),
                        (bd_ce_q, bd_rot_q, qlat_t, qT[pr]),
                    ):
                        ps_ce = pa([128, T], "ps_ce")
                        ps_rot = pa([128, T], "ps_rot")
                        for h2 in range(2):
                            sl = slice(h2 * 512, (h2 + 1) * 512)
                            nc.tensor.matmul(
                                ps_ce[:, sl], bd_ce[:], lat[:, pr, sl],
                                start=True, stop=True,
                            )
                            nc.tensor.matmul(
                                ps_rot[:, sl], bd_rot[:], lat[:, pr, sl],
                                start=True, stop=True,
                            )
                        nc.scalar.copy(dst[:], ps_ce[:])  # cast bf16
                        eng = nc.gpsimd if dst is qT[pr] else nc.vector
                        eng.tensor_mul(dst[:], dst[:], cc_t[:])
                        tmp = rtp.tile([128, T], BF16, tag="ropetmp", name="ropetmp")
                        nc.vector.tensor_mul(tmp[:], ps_rot[:], ss_t[:])
                        eng.tensor_add(dst[:], dst[:], tmp[:])

            # ---- phase 5: attention (+ per-pair normalization) ----
            with tc.tile_pool(name="pt", bufs=3) as ptp, tc.tile_pool(
                name="rbp", bufs=3
            ) as rbp:
                for pr in range(PAIRS):
                    ps_o = psO.tile([65, 2 * T], F32, tag="ps_o", name="ps_o")
                    for kt in range(KT):
                        koff = kt * 128
                        for qc in range(QC):
                            qlo = max(qc * 512, koff)
                            qhi = (qc + 1) * 512
                            if qhi <= koff:
                                continue
                            ncols = qhi - qlo
                            ps_s = pa([128, 2, 512], "ps_s")
                            for h2 in range(2):
                                dsl = slice(h2 * 64, (h2 + 1) * 64)
                                nc.tensor.matmul(
                                    ps_s[:, h2, 0:ncols],
                                    kT[pr][dsl, koff : koff + 128],
                                    qT[pr][dsl, qlo:qhi],
                                    start=True,
                                    stop=True,
                                )
                            pt = ptp.tile([128, 2, 512], BF16, tag="pt", name="pt")
                            nc.scalar.activation(
                                pt[:, :, 0:ncols],
                                ps_s[:, :, 0:ncols],
                                mybir.ActivationFunctionType.Exp,
                                scale=0.125,
                            )
                            if qlo == koff:  # diagonal 128-block lives here
                                nc.gpsimd.tensor_mul(
                                    pt[:, :, 0:128], pt[:, :, 0:128], tri2_t[:]
                                )
                            stop_kt = qc * 4 + 3
                            for h2 in range(2):
                                nc.tensor.matmul(
                                    ps_o[:, h2 * T + qlo : h2 * T + qhi],
                                    v_g[:, kt, pr * 2 + h2, :],
                                    pt[:, h2, 0:ncols],
                                    start=(kt == 0),
                                    stop=(kt == stop_kt),
                                )
                    nc.scalar.copy(
                        otu[:, pr * 2 : pr * 2 + 2, :],
                        ps_o[:].rearrange("p (h t) -> p h t", h=2),
                    )
                    # normalization for this pair, overlapped with next pair
                    r8b = rbp.tile([2, T], BF16, tag="r8b", name="r8b")
                    nc.sync.dma_start(
                        out=r8b[:],
                        in_=otu[64:65, pr * 2 : pr * 2 + 2, :],
                    )
                    r8f = rbp.tile([2, T], F32, tag="r8f", name="r8f")
                    nc.vector.tensor_copy(r8f[:], r8b[:])
                    nc.vector.reciprocal_approx_fast(out=r8f[:], in_=r8f[:])
                    r8c = rbp.tile([2, T], BF16, tag="r8c", name="r8c")
                    nc.vector.tensor_copy(r8c[:], r8f[:])
                    for h2 in range(2):
                        rsrc = r8c[h2 : h2 + 1, :]
                        src_b = bass.AP(
                            tensor=rsrc.tensor,
                            offset=rsrc.offset,
                            ap=[list(rsrc.ap[0]), [0, 64], list(rsrc.ap[1])],
                        )
                        rb = rbp.tile([64, T], BF16, tag="rb", name="rb")
                        nc.sync.dma_start(out=rb[:], in_=src_b)
                        nc.vector.tensor_mul(
                            otr[pr][h2 * 64 : (h2 + 1) * 64, :],
                            otu[0:64, pr * 2 + h2, :],
                            rb[:],
                        )

            # ---- phase 6: c_proj ----
            with tc.tile_pool(name="yout", bufs=3) as yop:
                for qt in range(8):
                    ps = pa([128, C], "ps_y")
                    for fp in range(PAIRS):
                        for half in range(2):
                            nc.tensor.matmul(
                                ps[:, half * 512 : (half + 1) * 512],
                                otr[fp][:, qt * 128 : (qt + 1) * 128],
                                wcr[fp][:, half * 512 : (half + 1) * 512],
                                start=(fp == 0),
                                stop=(fp == PAIRS - 1),
                            )
                    ysb = yop.tile([128, C], F32, tag="ysb", name="ysb")
                    nc.vector.tensor_copy(ysb[:], ps[:])
                    nc.sync.dma_start(
                        out=y_d[qt * 128 : (qt + 1) * 128, :], in_=ysb[:]
                    )

    nc.finalize()
    return nc


@functools.lru_cache(maxsize=1)
def _get_nc():
    return _build_nc()


@functools.lru_cache(maxsize=1)
def _host_tables():
    cos, sin = _rope_tables()
    cc = np.ones((128, T), dtype=np.float32)
    cc[32:64] = cos
    cc[96:128] = cos
    ss = np.zeros((128, T), dtype=np.float32)
    ss[32:64] = sin
    ss[96:128] = sin
    tri = np.tril(np.ones((128, 128), dtype=np.float32)).T  # [k, q]: 1 if k<=q
    tri2 = np.stack([tri, tri], axis=1)  # [128, 2, 128]
    ident = np.eye(128, dtype=np.float32)
    return cc, ss, tri2, ident


def _bf16(a):
    import ml_dtypes

    return np.asarray(a, dtype=ml_dtypes.bfloat16)


def kernel(x, Wq_down, Wk_down, Wv_down, Wq_up_c, Wq_up_e, Wk_up_c, Wk_up_e, Wv_up, Wc):
    x = np.asarray(x, dtype=np.float32)
    Wq_down = np.asarray(Wq_down, dtype=np.float32)
    Wk_down = np.asarray(Wk_down, dtype=np.float32)
    Wv_down = np.asarray(Wv_down, dtype=np.float32)
    Wc = np.asarray(Wc, dtype=np.float32)

    cc, ss, tri2, ident = _host_tables()
    tmask = np.triu(np.full((128, 128), -240.0, dtype=np.float32), 1)
    bd_ce_q = _blockdiag2(
        np.concatenate([np.asarray(Wq_up_c), np.asarray(Wq_up_e)], axis=1)
    ).astype(np.float32)
    bd_ce_k = _blockdiag2(
        np.concatenate([np.asarray(Wk_up_c), np.asarray(Wk_up_e)], axis=1)
    ).astype(np.float32)
    bd_rot_q = _rot_ext(np.asarray(Wq_up_e))
    bd_rot_k = _rot_ext(np.asarray(Wk_up_e))
    bd_v = _bf16(_blockdiag2(np.asarray(Wv_up)))

    nc = _get_nc()
    in_maps = []
    for core in range(NCORES):
        b, hh = core // 2, core % 2
        sl = slice(hh * 512, (hh + 1) * 512)
        in_maps.append(
            {
                "x": x[b],
                "wq": _bf16(Wq_down[:, sl]),
                "wk": _bf16(Wk_down[:, sl]),
                "wv": _bf16(Wv_down[:, sl]),
                "wc": _bf16(Wc[sl, :]),
                "bd_ce_q": _bf16(bd_ce_q),
                "bd_ce_k": _bf16(bd_ce_k),
                "bd_rot_q": _bf16(bd_rot_q),
                "bd_rot_k": _bf16(bd_rot_k),
                "bd_v": bd_v,
                "cc": _bf16(cc),
                "ss": _bf16(ss),
                "tri2": _bf16(tri2),
                "tmask": _bf16(tmask),
                "idbf": _bf16(ident),
                "ident": ident,
            }
        )

    kwargs = {}
    if TRACE:
        import shutil, os
        shutil.rmtree("/tmp/mla_trace", ignore_errors=True)
        os.makedirs("/tmp/mla_trace", exist_ok=True)
        kwargs = dict(trace=True, tmpdir="/tmp/mla_trace")
    res = run_bass_kernel_spmd(nc, in_maps, core_ids=list(range(NCORES)), **kwargs)
    if TRACE:
        kernel.last_exec_time_ns = res.exec_time_ns

    y = np.empty((B, T, C), dtype=np.float32)
    k_lat = np.empty((B, T, H, L), dtype=np.float32)
    v_lat = np.empty((B, T, H, L), dtype=np.float32)
    for b in range(B):
        r0, r1 = res.results[2 * b], res.results[2 * b + 1]
        y[b] = r0["y_part"] + r1["y_part"]
        for hh, r in ((0, r0), (1, r1)):
            hs = slice(hh * 8, hh * 8 + 8)
            k_lat[b, :, hs, :] = r["klatT"].reshape(8, 64, T).transpose(2, 0, 1)
            v_lat[b, :, hs, :] = r["vlatT"].reshape(8, 64, T).transpose(2, 0, 1)
    return y, k_lat, v_lat


# revision 36
# speedup vs baseline: 1.1015x; 1.0226x over previous
"""Trainium2 Bass kernel for MLA self-attention (nn_MLASelfAttentionWithMoBA).

Sharding: 8 cores = 4 batches x 2 head-halves (tensor parallel over heads).
Each core handles one batch element and 8 of the 16 heads:
  - down-projections x @ W*_down restricted to its heads' latent columns
  - per-head up-projections + partial RoPE (pair-mixing folded into a
    host-precomputed "rotated" copy of the up-projection weights)
  - full causal attention for its 8 heads, scores kept transposed [k, q]
    so A@V needs no P transposes; softmax denominators come from an extra
    ones-column appended to V; normalization is applied to the attention
    output before the c_proj
  - row-parallel c_proj with its 512 rows of Wc -> partial y
Host combines: y = y_half0 + y_half1 per batch; k_lat/v_lat are written
head-major-transposed [512, T] per core and transposed on the host.

Down-projection runs in f32r (tf32-like, full PE rate) so the k_lat/v_lat
outputs keep ~1e-4 precision; attention/up/c_proj operands are bf16/f32r.
"""

import functools

import numpy as np

import concourse.bass as bass
import concourse.tile as tile
from concourse import bacc, mybir
from concourse.bass_utils import run_bass_kernel_spmd

F32 = mybir.dt.float32
F32R = mybir.dt.float32r
BF16 = mybir.dt.bfloat16

B, T, C, H, L = 4, 1024, 1024, 16, 64
DH, DE = 64, 32
NCORES = 8
HPC = H // 2  # heads per core (8)
PAIRS = HPC // 2  # head pairs per core (4)
KT = T // 128  # key tiles (8)
QC = T // 512  # q chunks (2)

TRACE = False  # set by test.py to capture an NTFF profile


def _rope_tables():
    inv = 1.0 / (10000.0 ** (np.arange(0, DE, 2, dtype=np.float64) / DE))  # [16]
    t = np.arange(T, dtype=np.float64)
    freqs = np.outer(t, inv)  # [T, 16]
    emb = np.concatenate([freqs, freqs], axis=-1)  # [T, 32]
    cos = np.cos(emb).T.astype(np.float32)  # [32, T]
    sin = np.sin(emb).T.astype(np.float32)
    return cos, sin


def _rot_cols(w):
    """w: [L, DE]. Columns permuted/negated so lat@w_rot == rotate_half(lat@w)."""
    r = np.empty_like(w)
    r[:, 0::2] = -w[:, 1::2]
    r[:, 1::2] = w[:, 0::2]
    return r


def _blockdiag2(a):
    z = np.zeros_like(a)
    return np.block([[a, z], [z, a]])  # [2m, 2n]


def _rot_ext(w):
    """[128, 128] table: out rows 32-63 <- rot-proj(h0), rows 96-127 <- rot-proj(h1),
    other out rows 0 (so the rope add is one full-height op)."""
    r = _rot_cols(w)  # [64, 32]
    ext = np.zeros((128, 128), dtype=np.float32)
    ext[0:64, 32:64] = r
    ext[64:128, 96:128] = r
    return ext


def _build_nc():
    nc = bacc.Bacc("TRN2", target_bir_lowering=False, debug=False, num_devices=NCORES)

    x_d = nc.dram_tensor("x", [T, C], F32, kind="ExternalInput")
    wq_d = nc.dram_tensor("wq", [C, 512], BF16, kind="ExternalInput")
    wk_d = nc.dram_tensor("wk", [C, 512], BF16, kind="ExternalInput")
    wv_d = nc.dram_tensor("wv", [C, 512], BF16, kind="ExternalInput")
    wc_d = nc.dram_tensor("wc", [512, C], BF16, kind="ExternalInput")
    bd_ce_q_d = nc.dram_tensor("bd_ce_q", [128, 128], BF16, kind="ExternalInput")
    bd_ce_k_d = nc.dram_tensor("bd_ce_k", [128, 128], BF16, kind="ExternalInput")
    bd_rot_q_d = nc.dram_tensor("bd_rot_q", [128, 128], BF16, kind="ExternalInput")
    bd_rot_k_d = nc.dram_tensor("bd_rot_k", [128, 128], BF16, kind="ExternalInput")
    bd_v_d = nc.dram_tensor("bd_v", [128, 128], BF16, kind="ExternalInput")
    cc_d = nc.dram_tensor("cc", [128, T], BF16, kind="ExternalInput")
    ss_d = nc.dram_tensor("ss", [128, T], BF16, kind="ExternalInput")
    tri2_d = nc.dram_tensor("tri2", [128, 2, 128], BF16, kind="ExternalInput")
    tmask_d = nc.dram_tensor("tmask", [128, 128], BF16, kind="ExternalInput")
    idbf_d = nc.dram_tensor("idbf", [128, 128], BF16, kind="ExternalInput")
    ident_d = nc.dram_tensor("ident", [128, 128], F32, kind="ExternalInput")

    y_d = nc.dram_tensor("y_part", [T, C], F32, kind="ExternalOutput")
    klat_d = nc.dram_tensor("klatT", [512, T], BF16, kind="ExternalOutput")
    vlat_d = nc.dram_tensor("vlatT", [512, T], BF16, kind="ExternalOutput")

    with tile.TileContext(nc) as tc:
        with tc.tile_pool(name="const", bufs=1) as const, tc.tile_pool(
            name="persist", bufs=1
        ) as persist, tc.tile_pool(
            name="psA", bufs=2, space="PSUM"
        ) as psA, tc.tile_pool(name="psO", bufs=1, space="PSUM") as psO:

            def pa(shape, name):
                padded = [128] + list(shape[1:-1]) + [T // (int(np.prod(shape[1:-1])) or 1)]
                return psA.tile(shape, F32, tag="A", name=name, padded_shape=padded)

            # ---- constants ----
            cc_t = const.tile([128, T], BF16, tag="cc")
            nc.sync.dma_start(out=cc_t[:], in_=cc_d[:])
            ss_t = const.tile([128, T], BF16, tag="ss")
            nc.sync.dma_start(out=ss_t[:], in_=ss_d[:])
            tri2_t = const.tile([128, 2, 128], BF16, tag="tri2")
            nc.sync.dma_start(out=tri2_t[:], in_=tri2_d[:])
            id_t = const.tile([128, 128], F32, tag="ident")
            nc.sync.dma_start(out=id_t[:], in_=ident_d[:])
            bd_ce_q = const.tile([128, 128], BF16, tag="bd_ce_q")
            nc.sync.dma_start(out=bd_ce_q[:], in_=bd_ce_q_d[:])
            bd_ce_k = const.tile([128, 128], BF16, tag="bd_ce_k")
            nc.sync.dma_start(out=bd_ce_k[:], in_=bd_ce_k_d[:])
            bd_rot_q = const.tile([128, 128], BF16, tag="bd_rot_q")
            nc.sync.dma_start(out=bd_rot_q[:], in_=bd_rot_q_d[:])
            bd_rot_k = const.tile([128, 128], BF16, tag="bd_rot_k")
            nc.sync.dma_start(out=bd_rot_k[:], in_=bd_rot_k_d[:])
            bd_v_t = const.tile([128, 128], BF16, tag="bd_v")
            nc.sync.dma_start(out=bd_v_t[:], in_=bd_v_d[:])
            tmask_t = const.tile([128, 128], BF16, tag="tmask")
            nc.sync.dma_start(out=tmask_t[:], in_=tmask_d[:])
            idbf_t = const.tile([128, 128], BF16, tag="idbf")
            nc.sync.dma_start(out=idbf_t[:], in_=idbf_d[:])

            # PE warmup burst so the HAM clock-gate reaches K=8 early
            wsrc = const.tile([128, 512], BF16, tag="wsrc")
            nc.vector.memset(wsrc[:], 1.0)
            for wi in range(8):
                pw = pa([128, 512], "pw")
                nc.tensor.matmul(pw[:], wsrc[:, 0:128], wsrc[:], start=True, stop=True)

            # Wc: load early so c_proj never waits on it
            wcr = []
            for fp in range(PAIRS):
                wt = persist.tile([128, C], BF16, tag=f"wcr{fp}", name=f"wcr{fp}")
                nc.scalar.dma_start(out=wt[:], in_=wc_d[fp * 128 : (fp + 1) * 128, :])
                wcr.append(wt)

            # persistent intermediates (bf16 to fit SBUF)
            qlat_t = persist.tile([128, PAIRS, T], BF16, tag="qlat")
            klat_t = persist.tile([128, PAIRS, T], BF16, tag="klat")
            vlat_t = persist.tile([128, PAIRS, T], BF16, tag="vlat")
            v_g = persist.tile([128, KT, HPC, 65], BF16, tag="v_g")
            kT = [persist.tile([128, T], BF16, tag=f"kT{p}", name=f"kT{p}") for p in range(PAIRS)]
            qT = [persist.tile([128, T], BF16, tag=f"qT{p}", name=f"qT{p}") for p in range(PAIRS)]
            otu = persist.tile([65, HPC, T], BF16, tag="otu")
            otr = [persist.tile([128, T], BF16, tag=f"otr{p}", name=f"otr{p}") for p in range(PAIRS)]

            # ---- phase 1: x load + transpose (PE) ----
            with tc.tile_pool(name="xT", bufs=1) as xtp:
                xT = xtp.tile([128, KT, T], BF16, tag="xT")
                with tc.tile_pool(name="xrow", bufs=3) as xrp:
                    for ti in range(8):
                        xrow = xrp.tile([128, C], F32)
                        nc.sync.dma_start(
                            out=xrow[:], in_=x_d[ti * 128 : (ti + 1) * 128, :]
                        )
                        for g in range(2):  # ci groups of 4
                            ps = pa([128, 512], "ps_tr")
                            for j in range(4):
                                ci = g * 4 + j
                                nc.tensor.transpose(
                                    ps[:, j * 128 : (j + 1) * 128],
                                    xrow[:, ci * 128 : (ci + 1) * 128],
                                    id_t[:],
                                )
                            dst = xT[:, g * 4 : g * 4 + 4, ti * 128 : (ti + 1) * 128]
                            src = ps[:].rearrange("p (c t) -> p c t", c=4)
                            if (ti + g) % 2 == 0:
                                nc.vector.tensor_copy(dst, src)
                            else:
                                nc.scalar.copy(dst, src)

                # ---- phase 2: down-projections ----
                with tc.tile_pool(name="wr", bufs=10) as wrp:
                    for w_d, lat, out_d in (
                        (wq_d, qlat_t, None),
                        (wk_d, klat_t, klat_d),
                        (wv_d, vlat_t, vlat_d),
                    ):
                        wr = []
                        for ci in range(8):
                            wt = wrp.tile([128, 512], BF16, tag="wt", name="wt")
                            nc.scalar.dma_start(
                                out=wt[:], in_=w_d[ci * 128 : (ci + 1) * 128, :]
                            )
                            wr.append(wt)
                        for lt in range(PAIRS):
                            ps = pa([128, T], "ps_dn")
                            for ci in range(8):
                                for h2 in range(2):
                                    nc.tensor.matmul(
                                        ps[:, h2 * 512 : (h2 + 1) * 512],
                                        wr[ci][:, lt * 128 : (lt + 1) * 128],
                                        xT[:, ci, h2 * 512 : (h2 + 1) * 512],
                                        start=(ci == 0),
                                        stop=(ci == 7),
                                    )
                            nc.vector.tensor_copy(lat[:, lt, :], ps[:])
                            if out_d is not None:
                                nc.sync.dma_start(
                                    out=out_d[lt * 128 : (lt + 1) * 128, :],
                                    in_=lat[:, lt, :],
                                )

            # ---- phase 3: v up-projection ----
            nc.gpsimd.memset(v_g[:, :, :, 64:65], 1.0)
            for tci in range(KT):
                ps = pa([128, 512], "ps_v")
                for p in range(PAIRS):
                    nc.tensor.matmul(
                        ps[:, p * 128 : (p + 1) * 128],
                        vlat_t[:, p, tci * 128 : (tci + 1) * 128],
                        bd_v_t[:],
                        start=True,
                        stop=True,
                    )
                nc.scalar.copy(
                    v_g[:, tci, :, 0:64],
                    ps[:].rearrange("p (h d) -> p h d", d=64),
                )

            # ---- phase 4: q/k up-projection + rope ----
            with tc.tile_pool(name="ropetmp", bufs=2) as rtp:
                for pr in range(PAIRS):
                    for bd_ce, bd_rot, lat, dst in (
                        (bd_ce_k, bd_rot_k, klat_t, kT[pr]),
                        (bd_ce_q, bd_rot_q, qlat_t, qT[pr]),
                    ):
                        ps_ce = pa([128, T], "ps_ce")
                        ps_rot = pa([128, T], "ps_rot")
                        for h2 in range(2):
                            sl = slice(h2 * 512, (h2 + 1) * 512)
                            nc.tensor.matmul(
                                ps_ce[:, sl], bd_ce[:], lat[:, pr, sl],
                                start=True, stop=True,
                            )
                            nc.tensor.matmul(
                                ps_rot[:, sl], bd_rot[:], lat[:, pr, sl],
                                start=True, stop=True,
                            )
                        nc.scalar.copy(dst[:], ps_ce[:])  # cast bf16
                        eng = nc.gpsimd if dst is qT[pr] else nc.vector
                        eng.tensor_mul(dst[:], dst[:], cc_t[:])
                        tmp = rtp.tile([128, T], BF16, tag="ropetmp", name="ropetmp")
                        nc.vector.tensor_mul(tmp[:], ps_rot[:], ss_t[:])
                        eng.tensor_add(dst[:], dst[:], tmp[:])

            # ---- phase 5: attention (+ per-pair normalization) ----
            with tc.tile_pool(name="pt", bufs=3) as ptp, tc.tile_pool(
                name="rbp", bufs=3
            ) as rbp:
                for pr in range(PAIRS):
                    ps_o = psO.tile([65, 2 * T], F32, tag="ps_o", name="ps_o")
                    for kt in range(KT):
                        koff = kt * 128
                        for qc in range(QC):
                            qlo = max(qc * 512, koff)
                            qhi = (qc + 1) * 512
                            if qhi <= koff:
                                continue
                            ncols = qhi - qlo
                            ps_s = pa([128, 2, 512], "ps_s")
                            for h2 in range(2):
                                dsl = slice(h2 * 64, (h2 + 1) * 64)
                                nc.tensor.matmul(
                                    ps_s[:, h2, 0:ncols],
                                    kT[pr][dsl, koff : koff + 128],
                                    qT[pr][dsl, qlo:qhi],
                                    start=True,
                                    stop=True,
                                )
                            pt = ptp.tile([128, 2, 512], BF16, tag="pt", name="pt")
                            nc.scalar.activation(
                                pt[:, :, 0:ncols],
                                ps_s[:, :, 0:ncols],
                                mybir.ActivationFunctionType.Exp,
                                scale=0.125,
                            )
                            if qlo == koff:  # diagonal 128-block lives here
                                nc.gpsimd.tensor_mul(
                                    pt[:, :, 0:128], pt[:, :, 0:128], tri2_t[:]
                                )
                            stop_kt = qc * 4 + 3
                            for h2 in range(2):
                                nc.tensor.matmul(
                                    ps_o[:, h2 * T + qlo : h2 * T + qhi],
                                    v_g[:, kt, pr * 2 + h2, :],
                                    pt[:, h2, 0:ncols],
                                    start=(kt == 0),
                                    stop=(kt == stop_kt),
                                )
                    nc.scalar.copy(
                        otu[:, pr * 2 : pr * 2 + 2, :],
                        ps_o[:].rearrange("p (h t) -> p h t", h=2),
                    )
                    # normalization for this pair, overlapped with next pair
                    r8b = rbp.tile([2, T], BF16, tag="r8b", name="r8b")
                    nc.sync.dma_start(
                        out=r8b[:],
                        in_=otu[64:65, pr * 2 : pr * 2 + 2, :],
                    )
                    r8f = rbp.tile([2, T], F32, tag="r8f", name="r8f")
                    nc.vector.tensor_copy(r8f[:], r8b[:])
                    nc.vector.reciprocal_approx_fast(out=r8f[:], in_=r8f[:])
                    r8c = rbp.tile([2, T], BF16, tag="r8c", name="r8c")
                    nc.vector.tensor_copy(r8c[:], r8f[:])
                    for h2 in range(2):
                        rsrc = r8c[h2 : h2 + 1, :]
                        src_b = bass.AP(
                            tensor=rsrc.tensor,
                            offset=rsrc.offset,
                            ap=[list(rsrc.ap[0]), [0, 64], list(rsrc.ap[1])],
                        )
                        rb = rbp.tile([64, T], BF16, tag="rb", name="rb")
                        nc.sync.dma_start(out=rb[:], in_=src_b)
                        nc.vector.tensor_mul(
                            otr[pr][h2 * 64 : (h2 + 1) * 64, :],
                            otu[0:64, pr * 2 + h2, :],
                            rb[:],
                        )

            # ---- phase 6: c_proj ----
            with tc.tile_pool(name="yout", bufs=3) as yop:
                for qt in range(8):
                    ps = pa([128, C], "ps_y")
                    for fp in range(PAIRS):
                        for half in range(2):
                            nc.tensor.matmul(
                                ps[:, half * 512 : (half + 1) * 512],
                                otr[fp][:, qt * 128 : (qt + 1) * 128],
                                wcr[fp][:, half * 512 : (half + 1) * 512],
                                start=(fp == 0),
                                stop=(fp == PAIRS - 1),
                            )
                    ysb = yop.tile([128, C], F32, tag="ysb", name="ysb")
                    nc.vector.tensor_copy(ysb[:], ps[:])
                    nc.sync.dma_start(
                        out=y_d[qt * 128 : (qt + 1) * 128, :], in_=ysb[:]
                    )

    nc.finalize()
    return nc


@functools.lru_cache(maxsize=1)
def _get_nc():
    return _build_nc()


@functools.lru_cache(maxsize=1)
def _host_tables():
    cos, sin = _rope_tables()
    cc = np.ones((128, T), dtype=np.float32)
    cc[32:64] = cos
    cc[96:128] = cos
    ss = np.zeros((128, T), dtype=np.float32)
    ss[32:64] = sin
    ss[96:128] = sin
    tri = np.tril(np.ones((128, 128), dtype=np.float32)).T  # [k, q]: 1 if k<=q
    tri2 = np.stack([tri, tri], axis=1)  # [128, 2, 128]
    ident = np.eye(128, dtype=np.float32)
    return cc, ss, tri2, ident


def _bf16(a):
    import ml_dtypes

    return np.asarray(a, dtype=ml_dtypes.bfloat16)


def kernel(x, Wq_down, Wk_down, Wv_down, Wq_up_c, Wq_up_e, Wk_up_c, Wk_up_e, Wv_up, Wc):
    x = np.asarray(x, dtype=np.float32)
    Wq_down = np.asarray(Wq_down, dtype=np.float32)
    Wk_down = np.asarray(Wk_down, dtype=np.float32)
    Wv_down = np.asarray(Wv_down, dtype=np.float32)
    Wc = np.asarray(Wc, dtype=np.float32)

    cc, ss, tri2, ident = _host_tables()
    tmask = np.triu(np.full((128, 128), -240.0, dtype=np.float32), 1)
    bd_ce_q = _blockdiag2(
        np.concatenate([np.asarray(Wq_up_c), np.asarray(Wq_up_e)], axis=1)
    ).astype(np.float32)
    bd_ce_k = _blockdiag2(
        np.concatenate([np.asarray(Wk_up_c), np.asarray(Wk_up_e)], axis=1)
    ).astype(np.float32)
    bd_rot_q = _rot_ext(np.asarray(Wq_up_e))
    bd_rot_k = _rot_ext(np.asarray(Wk_up_e))
    bd_v = _bf16(_blockdiag2(np.asarray(Wv_up)))

    nc = _get_nc()
    in_maps = []
    for core in range(NCORES):
        b, hh = core // 2, core % 2
        sl = slice(hh * 512, (hh + 1) * 512)
        in_maps.append(
            {
                "x": x[b],
                "wq": _bf16(Wq_down[:, sl]),
                "wk": _bf16(Wk_down[:, sl]),
                "wv": _bf16(Wv_down[:, sl]),
                "wc": _bf16(Wc[sl, :]),
                "bd_ce_q": _bf16(bd_ce_q),
                "bd_ce_k": _bf16(bd_ce_k),
                "bd_rot_q": _bf16(bd_rot_q),
                "bd_rot_k": _bf16(bd_rot_k),
                "bd_v": bd_v,
                "cc": _bf16(cc),
                "ss": _bf16(ss),
                "tri2": _bf16(tri2),
                "tmask": _bf16(tmask),
                "idbf": _bf16(ident),
                "ident": ident,
            }
        )

    kwargs = {}
    if TRACE:
        import shutil, os
        shutil.rmtree("/tmp/mla_trace", ignore_errors=True)
        os.makedirs("/tmp/mla_trace", exist_ok=True)
        kwargs = dict(trace=True, tmpdir="/tmp/mla_trace")
    res = run_bass_kernel_spmd(nc, in_maps, core_ids=list(range(NCORES)), **kwargs)
    if TRACE:
        kernel.last_exec_time_ns = res.exec_time_ns

    y = np.empty((B, T, C), dtype=np.float32)
    k_lat = np.empty((B, T, H, L), dtype=np.float32)
    v_lat = np.empty((B, T, H, L), dtype=np.float32)
    for b in range(B):
        r0, r1 = res.results[2 * b], res.results[2 * b + 1]
        y[b] = r0["y_part"] + r1["y_part"]
        for hh, r in ((0, r0), (1, r1)):
            hs = slice(hh * 8, hh * 8 + 8)
            k_lat[b, :, hs, :] = (
                r["klatT"].astype(np.float32).reshape(8, 64, T).transpose(2, 0, 1)
            )
            v_lat[b, :, hs, :] = (
                r["vlatT"].astype(np.float32).reshape(8, 64, T).transpose(2, 0, 1)
            )
    return y, k_lat, v_lat
